# revision 60
# baseline (speedup 1.0000x reference)
"""Trainium2 Bass kernel for nn_AttnFPN (conv pyramid + 4-layer transformer
decoder with banded self-attention + dense cross-attention over a conv memory).

Sharding: 8 cores = 2 batches x 4 window-quarters of the concatenated pyramid
row space (1920 rows). Each core computes the full conv pyramid for its batch,
selects a 512-row window (480 owned rows + 16-row halo each side) via a
one-hot selection matmul, runs all 4 decoder layers on the window (halo
shrink absorbs the banded self-attention's +-4 reach per layer), and emits its
480 owned rows. The host assembles the [B, 256, 1920] output.

On-chip layout is feature-major throughout: activations live as X^T
[d on partitions (2x128 chunks), rows on free dim]. All dense projections run
as float32r matmuls (full PE rate at >=256 moving columns, ~fp32 precision);
attention score/AV matmuls run in bf16. K biases are dropped (softmax is
invariant to per-query logit shifts); V biases are folded into the output
projection bias on the host (softmax weights sum to 1). Softmax denominators
come from 32-wide ones columns interleaved in the V stationary tiles.
Cross-attention K/V for layer l+1 is computed via a filler queue drained
during layernorm gaps and cross-attention of layer l to keep the PE busy.
The conv pyramid runs fully in bf16; x, selection one-hots, and conv weights
ship as bf16. LayerNorm 1/sigma uses exp(-0.5*ln(var+eps)) so the Activation
engine stays on a single ln/exp/square table (no table reloads). Matmul PSUM
outputs always start on 2KB bank boundaries (hardware requirement)."""
import os
import sys

for _p in ('/opt/trn_rl_repo', os.path.expanduser('~/.axon_site/_ro/trn_rl_repo')):
    if os.path.isdir(_p) and _p not in sys.path:
        sys.path.insert(0, _p)

import ml_dtypes
import numpy as np

import concourse.bass as bass
import concourse.mybir as mybir
import concourse.tile as tile
from concourse import bacc
from concourse.bass_utils import run_bass_kernel_spmd
from concourse.masks import make_identity

F32 = mybir.dt.float32
F32R = mybir.dt.float32r
BF16 = mybir.dt.bfloat16
I32 = mybir.dt.int32
FP8 = mybir.dt.float8e4
PM = mybir.MatmulPerfMode
AF = mybir.ActivationFunctionType
OP = mybir.AluOpType

# problem constants
B, CIN, COUT, T, NLV, NLY, H, DFF, KBAND = 2, 512, 256, 2048, 4, 4, 8, 1024, 9
HD = COUT // H           # 32
RW = 512                 # per-core window rows
OWN = 480
HALO = 16
LVL_SIZES = [1024, 512, 256, 128]
LVL_STARTS = [0, 1024, 1536, 1792]
TOT = 1920
NBLK = TOT // 128        # 15 row-blocks of the concat pyramid
XP = 2056                # padded x length (col j holds x[:, j-1], col 0 = zero)
# self-attn subtiles: (q_start, q_len, k_start) window-local
SUBTILES = [(0, 120, 0), (120, 120, 116), (240, 120, 236), (360, 120, 356), (480, 32, 384)]

# ---------------------------------------------------------------------------
# device program
# ---------------------------------------------------------------------------


def _build_nc():
    nc = bacc.Bacc("TRN2", target_bir_lowering=False, debug=False, num_devices=8)

    def din(name, shape, dt=F32R):
        return nc.dram_tensor(name, list(shape), dt, kind="ExternalInput")

    t_x = din("xp", [4, 128, XP], BF16)           # x padded, feature chunks
    t_ssel = din("ssel", [NBLK, 128, RW], BF16)   # one-hot selection
    t_pe = din("pe", [128, 2, RW], BF16)          # PE slice, chunked
    t_smask = din("smask", [5, 128, 256], BF16)   # additive self masks per subtile
    # conv weights (lhsT layouts [i-chunk 128, o]), batched for single DMAs
    t_w1x1 = din("w1x1", [4, 128, 256], BF16)     # [cc, p, f]
    t_wn0 = din("wn0", [12, 128, 256], BF16)      # [cc*3+tap, p, f]
    t_wnk = din("wnk", [18, 128, 256], BF16)      # [lvl*6+cc*3+tap, p, f]
    t_cnb = din("cnb", [128, 10], F32)            # conv_b (2) + neck biases (4x2)
    # per-layer transformer weights: one qkv/out blob + one ffn blob + one
    # bias/ln blob per layer (single DMA each)
    t_wqkv = din("wqkv", [NLY, 2, 128, 1536])     # sa_w|sa_ow|ca_qw|ca_ow
    t_wff = din("wff", [NLY, 128, 4096])          # ff1w (2x1024) | ff2w (8x256)
    t_lb = din("lbias", [NLY, 128, 30], F32)      # b_sa|b_sao|b_caq|b_cao|b_ff1|b_ff2|lngp
    t_ca_kw = din("ca_kw", [NLY, 2, 128, 256], BF16)
    t_ca_vw = din("ca_vw", [NLY, 2, 128, 512], BF16)  # augmented: v at cols [64h,64h+32)
    t_out = nc.dram_tensor("out", [128, 2, RW], F32R, kind="ExternalOutput")

    with tile.TileContext(nc) as tc:
        _emit(nc, tc, locals())
    nc.compile()
    return nc


_MM_TAGS = []
_PHASE = ['?']


def _tag_matmuls(nc):
    """Profiling aid (KERN_MMPROF=1): record a phase label per emitted matmul."""
    real = nc.tensor.matmul
    real_tr = nc.tensor.transpose

    def mm(*a, **k):
        _MM_TAGS.append(_PHASE[0])
        return real(*a, **k)

    def tr(*a, **k):
        _MM_TAGS.append(_PHASE[0] + ':tr')
        return real_tr(*a, **k)

    nc.tensor.matmul = mm
    nc.tensor.transpose = tr


def _emit(nc, tc, t):
    from contextlib import ExitStack
    if os.environ.get('KERN_MMPROF'):
        _tag_matmuls(nc)
    ctx = ExitStack()
    with ctx:
        ctx.enter_context(nc.allow_low_precision(
            reason="float32r keeps ~fp32 precision at full PE rate"))
        P = 128
        persist = ctx.enter_context(tc.tile_pool(name="persist", bufs=1))
        state = ctx.enter_context(tc.tile_pool(name="state", bufs=3))
        big = ctx.enter_context(tc.tile_pool(name="big", bufs=1))
        kvp = ctx.enter_context(tc.tile_pool(name="kvp", bufs=2))
        wb = ctx.enter_context(tc.tile_pool(name="wb", bufs=1))
        work = ctx.enter_context(tc.tile_pool(name="work", bufs=2))
        stats = ctx.enter_context(tc.tile_pool(name="stats", bufs=1))
        act = ctx.enter_context(tc.tile_pool(name="act", bufs=1))
        epool = ctx.enter_context(tc.tile_pool(name="epool", bufs=4))
        psc = ctx.enter_context(tc.tile_pool(name="psc", bufs=2, space="PSUM"))
        pav = ctx.enter_context(tc.tile_pool(name="pav", bufs=2, space="PSUM"))
        pms = ctx.enter_context(tc.tile_pool(name="pms", bufs=2, space="PSUM"))

        def stride2(ap3, cc, s, w):
            return ap3[:, cc, s:s + 2 * w].rearrange("p (n two) -> p two n", two=2)[:, 0, :]

        # ---- constants ----
        # gpsimd memset cannot emit float32r; route f32r constants through DVE
        ident_b = persist.tile([P, P], BF16)
        invn = persist.tile([P, P], F32R)
        invn_b = persist.tile([P, P], BF16)
        with tc.tile_pool(name="idtmp", bufs=1) as idtmp:
            ident_f = idtmp.tile([P, P], F32)
            make_identity(nc, ident_f[:])
            nc.vector.tensor_copy(ident_b[:], ident_f[:])
            nc.vector.tensor_scalar(out=invn[:], in0=ident_f[:], scalar1=0.0,
                                    scalar2=1.0 / COUT, op0=OP.mult, op1=OP.add)
            nc.vector.tensor_scalar(out=invn_b[:], in0=ident_f[:], scalar1=0.0,
                                    scalar2=1.0 / COUT, op0=OP.mult, op1=OP.add)

        # pe_sb/smask DMAs issued later (after the pyramid weight loads) so
        # the conv matmuls' weights head the HWDGE queue
        pe_sb = persist.tile([P, 2, RW], BF16)
        smask_sb = persist.tile([P, 5, 256], BF16)
        cexp = persist.tile([P, 1], F32)
        nc.gpsimd.memset(cexp[:], -2.0)

        memT = persist.tile([P, 2, T], BF16)

        _nlayers = int(os.environ.get('KERN_NLAYERS', str(NLY)))

        # per-layer K/V for cross attention, computed as drainable filler
        # closures so the PE stays busy during layernorm gaps.
        kv_state = {}

        def make_kv_fillers(l):
            """Return a list of closures; each emits one chunk of the cross
            K/V computation for layer l."""
            st = {}
            fillers = []

            def k_chunk(oc, tp2):
                def go():
                    _ph = _PHASE[0]
                    _PHASE[0] = 'kv'
                    if 'KT' not in st:
                        st['KT'] = kvp.tile([P, 2, T], BF16, tag="KT", name=f"KT{l}")
                        kv_state[l] = st
                    if 'wk' not in st:
                        st['wk'] = wb.tile([P, 2, 256], BF16, tag="wk", bufs=2,
                                           name=f"wk{l}")
                        nc.sync.dma_start(out=st['wk'][:],
                                          in_=t['t_ca_kw'][l].rearrange("c p f -> p c f"))
                    wk = st['wk'][:, :, 128 * oc:128 * (oc + 1)]
                    ps = psc.tile([P, 1024], F32, tag="sc", name=f"kps{l}_{oc}_{tp2}")
                    for half in range(2):
                        tck = 2 * tp2 + half
                        for ic in range(2):
                            nc.tensor.matmul(ps[:, 512 * half:512 * (half + 1)],
                                             wk[:, ic, :],
                                             memT[:, ic, 512 * tck:512 * (tck + 1)],
                                             start=(ic == 0), stop=(ic == 1))
                    nc.scalar.activation(out=st['KT'][:, oc, 1024 * tp2:1024 * (tp2 + 1)],
                                         in_=ps[:], func=AF.Copy)
                    _PHASE[0] = _ph
                return go

            def v_head(kc):
                def go():
                    _ph = _PHASE[0]
                    _PHASE[0] = 'kv'
                    if 'Vp' not in st:
                        st['Vp'] = kvp.tile([P, 16, 512], FP8, tag="Vp", name=f"Vp{l}")
                        kv_state[l] = st
                        # ones columns for the softmax denominator trick
                        vv = st['Vp'][:].rearrange("p k (h t d) -> p k h t d", h=8, t=2)
                        nc.gpsimd.memset(vv[:, :, :, 1, :], 1.0)
                        st['wv'] = wb.tile([P, 2, 512], BF16, tag="wv", bufs=2,
                                           name=f"wv{l}")
                        nc.sync.dma_start(out=st['wv'][:],
                                          in_=t['t_ca_vw'][l].rearrange("c p f -> p c f"))
                    ps = pav.tile([P, 512], F32, tag="av", name=f"vps{l}_{kc}")
                    for ic in range(2):
                        nc.tensor.matmul(ps[:], memT[:, ic, 128 * kc:128 * (kc + 1)],
                                         st['wv'][:, ic, :], start=(ic == 0), stop=(ic == 1))
                    # copy only the v columns; ones columns stay.  Act-engine
                    # copy: v_heads drain into LN windows where DVE/Pool run
                    # the rsqrt chain but Act idles.
                    src = ps[:].rearrange("p (h t d) -> p h t d", h=8, t=2)
                    dst = st['Vp'][:, kc, :].rearrange("p (h t d) -> p h t d", h=8, t=2)
                    nc.scalar.activation(out=dst[:, :, 0, :], in_=src[:, :, 0, :],
                                         func=AF.Copy)
                    _PHASE[0] = _ph
                return go

            for kc in range(16):
                fillers.append(v_head(kc))
            for oc in range(2):
                for tp2 in range(2):
                    fillers.append(k_chunk(oc, tp2))
            return fillers

        fill_q = []

        def drain(n):
            for _ in range(min(n, len(fill_q))):
                fill_q.pop(0)()

        def drain_tail(n):
            for _ in range(min(n, len(fill_q))):
                fill_q.pop()()


        # ================= pyramid =================
        _PHASE[0] = 'pyr'
        with tc.tile_pool(name="pyr", bufs=1) as pyr, \
             tc.tile_pool(name="wpyr", bufs=1) as wpyr, \
             tc.tile_pool(name="selw", bufs=2) as selw:
            # x arrives per feature-chunk (4 DMAs, queued first) so the mem
            # conv's cc=0 matmuls can start ~1.6us in instead of waiting for
            # the full 2.1MB transfer
            xT = pyr.tile([P, 4, XP], BF16)
            nc.sync.dma_start(out=xT[:, 0, :], in_=t['t_x'][0])
            w1x1 = wpyr.tile([P, 4, 256], BF16, tag="w1")
            nc.sync.dma_start(out=w1x1[:],
                              in_=t['t_w1x1'].ap().rearrange("c p f -> p c f"))
            for cc in range(1, 4):
                nc.sync.dma_start(out=xT[:, cc, :], in_=t['t_x'][cc])
            wn0 = wpyr.tile([P, 12, 256], BF16, tag="w0")
            nc.sync.dma_start(out=wn0[:],
                              in_=t['t_wn0'].ap().rearrange("c p f -> p c f"))
            wnk = wpyr.tile([P, 18, 256], BF16, tag="wk")
            nc.sync.dma_start(out=wnk[:],
                              in_=t['t_wnk'].ap().rearrange("c p f -> p c f"))
            cnb = pyr.tile([P, 10], F32)
            nc.sync.dma_start(out=cnb[:], in_=t['t_cnb'].ap())
            cb = cnb[:, 0:2]
            nb = cnb[:, 2:10].rearrange("p (l f) -> p l f", l=4)
            nc.sync.dma_start(out=pe_sb[:], in_=t['t_pe'].ap())
            nc.sync.dma_start(out=smask_sb[:],
                              in_=t['t_smask'].ap().rearrange("s p f -> p s f"))

            # mem = relu(1x1 conv), fc processed in pairs (2 psum slots)
            _PHASE[0] = 'pyr:mem'
            for oc in range(2):
                for fp in range(2):
                    pss = [pms.tile([P, 512], F32, tag="m", name=f"mempp{oc}_{fp}_{i2}") for i2 in range(2)]
                    for cc in range(4):
                        wsl = w1x1[:, cc, 128 * oc:128 * (oc + 1)]
                        for i, fc in enumerate((2 * fp, 2 * fp + 1)):
                            nc.tensor.matmul(pss[i][:], wsl,
                                             xT[:, cc, 1 + 512 * fc:1 + 512 * (fc + 1)],
                                             start=(cc == 0), stop=(cc == 3))
                    for i, fc in enumerate((2 * fp, 2 * fp + 1)):
                        nc.vector.tensor_scalar(out=memT[:, oc, 512 * fc:512 * (fc + 1)],
                                                in0=pss[i][:], scalar1=cb[:, 0 + oc:1 + oc],
                                                scalar2=0.0, op0=OP.add, op1=OP.max)

            # neck pyramid (feature-major, 1-col zero pad left)
            _PHASE[0] = 'pyr:neck'
            lvl_len = [1024, 512, 256, 128]
            zpad = pyr.tile([P, 2, 8], F32)
            nc.gpsimd.memset(zpad[:], 0.0)
            lbufs = []
            src_buf = None
            for lv in range(4):
                L = lvl_len[lv]
                lb = pyr.tile([P, 2, L + 8], BF16, tag=f"lb{lv}", name=f"lb{lv}")
                nc.vector.tensor_copy(lb[:, :, 0:1], zpad[:, :, 0:1])
                nc.vector.tensor_copy(lb[:, :, 1 + L:8 + L], zpad[:, :, 0:7])
                lbufs.append(lb)
                n_cc = 4 if lv == 0 else 2
                nfc = (L + 511) // 512
                for oc in range(2):
                    pss = [pms.tile([P, 512], F32, tag="m", name=f"cvp{lv}_{oc}_{i2}") for i2 in range(nfc)]
                    k = 0
                    for cc in range(n_cc):
                        for tap in range(3):
                            if lv == 0:
                                wsl = wn0[:, 3 * cc + tap, 128 * oc:128 * (oc + 1)]
                            else:
                                wsl = wnk[:, 6 * (lv - 1) + 3 * cc + tap, 128 * oc:128 * (oc + 1)]
                            for fc in range(nfc):
                                w = min(512, L - 512 * fc)
                                rhs = (stride2(xT, cc, 1024 * fc + tap, w) if lv == 0
                                       else stride2(src_buf, cc, 1024 * fc + tap, w))
                                nc.tensor.matmul(pss[fc][:, :w], wsl, rhs,
                                                 start=(k == 0), stop=(k == 3 * n_cc - 1))
                            k += 1
                    for fc in range(nfc):
                        w = min(512, L - 512 * fc)
                        nc.vector.tensor_scalar(out=lb[:, oc, 1 + 512 * fc:1 + 512 * fc + w],
                                                in0=pss[fc][:, :w], scalar1=nb[:, lv, oc:oc + 1],
                                                scalar2=0.0, op0=OP.add, op1=OP.max)
                src_buf = lb

            # transpose + selection, per feature chunk
            _PHASE[0] = 'pyr:sel'
            # layer 0's cross K/V computed as fillers inside the selection
            # phase, where the PE otherwise idles on the tr->copy->mm chain
            fill_q.extend(make_kv_fillers(0))
            f0 = state.tile([P, 2, RW], F32R, tag="fT")
            sselb = selw.tile([P, NBLK, RW], BF16, tag="sselb", bufs=1)
            nc.sync.dma_start(out=sselb[:],
                              in_=t['t_ssel'].ap().rearrange("b p w -> p b w"))
            blk_of = []
            for lv in range(4):
                for j in range(lvl_len[lv] // 128):
                    blk_of.append((lv, j))
            for dc in range(2):
                sel_ps = pms.tile([P, 512], F32, tag="m")

                def blk_prep(bb):
                    # two transposes per PSUM tile (cols 0 and 1024 bf16 =
                    # 2KB bank-aligned byte offsets), one paired fr copy
                    n = min(2, NBLK - bb)
                    tr_ps = psc.tile([P, 2048], BF16, tag="sc", name=f"trps{dc}_{bb}")
                    for i in range(n):
                        lv, j = blk_of[bb + i]
                        nc.tensor.transpose(tr_ps[:, 1024 * i:1024 * i + 128],
                                            lbufs[lv][:, dc, 1 + 128 * j:1 + 128 * (j + 1)],
                                            ident_b[:])
                    fr = selw.tile([P, 2, P], BF16, tag="frow", name=f"fr{dc}_{bb}")
                    trv = tr_ps[:].rearrange("p (b c) -> p b c", b=2)[:, 0:n, 0:128]
                    nc.vector.tensor_copy(fr[:, 0:n, :], trv)
                    return fr

                frp = blk_prep(0)
                for bb in range(0, NBLK, 2):
                    frn = blk_prep(bb + 2) if bb + 2 < NBLK else None
                    for i in range(min(2, NBLK - bb)):
                        b = bb + i
                        nc.tensor.matmul(sel_ps[:], frp[:, i, :], sselb[:, b, :],
                                         start=(b == 0), stop=(b == NBLK - 1))
                    frp = frn
                    drain(1)
                nc.vector.scalar_tensor_tensor(out=f0[:, dc, :], in0=sel_ps[:],
                                               scalar=float(np.sqrt(COUT)),
                                               in1=pe_sb[:, dc, :],
                                               op0=OP.mult, op1=OP.add)

        # ================= decoder layers =================
        # per-layer weight blobs, double-buffered so layer l+1's loads issue
        # during layer l; pool opened after the pyramid pools free their SBUF
        wlay = ctx.enter_context(tc.tile_pool(name="wlay", bufs=1))
        lw = {}

        def load_layer_weights(l):
            wq = wlay.tile([P, 2, 1536], F32R, tag="wqkv", bufs=2, name=f"wqkv{l}")
            nc.sync.dma_start(out=wq[:],
                              in_=t['t_wqkv'][l].rearrange("c p f -> p c f"))
            wf = wlay.tile([P, 4096], F32R, tag="wff", bufs=2, name=f"wff{l}")
            nc.sync.dma_start(out=wf[:], in_=t['t_wff'][l])
            lb = wlay.tile([P, 30], F32, tag="lbias", bufs=2, name=f"lbias{l}")
            nc.sync.dma_start(out=lb[:], in_=t['t_lb'][l])
            lw[l] = (wq, wf, lb)

        load_layer_weights(0)

        # QKV tiles are produced by the PREVIOUS layer's LN3 consumer (rows
        # half by half), so the projections fill the LN3 chain's PE window.
        qkv_pend = {}

        def emit_qkv(ll, wq_t, lb_t, src, h):
            _ph = _PHASE[0]
            _PHASE[0] = 'sa:qkv'
            if ll not in qkv_pend:
                qkv_pend[ll] = (act.tile([P, 2, RW], BF16, tag="QTs", name=f"QTs{ll}"),
                                act.tile([P, 2, RW], BF16, tag="KTs", name=f"KTs{ll}"),
                                act.tile([P, 2, RW], BF16, tag="VTs", name=f"VTs{ll}"))
            QTs_, KTs_, VTs_ = qkv_pend[ll]
            rs = slice(256 * h, 256 * (h + 1))
            for wi, dst in ((0, QTs_), (1, KTs_), (2, VTs_)):
                for oc in range(2):
                    wsl = wq_t[:, :, 256 * wi + 128 * oc:256 * wi + 128 * (oc + 1)]
                    ps = pms.tile([P, 512], F32, tag="m", name=f"qkv{ll}_{wi}_{oc}_{h}")
                    for ic in range(2):
                        nc.tensor.matmul(ps[:, 0:256], wsl[:, ic, :], src[:, ic, rs],
                                         start=(ic == 0), stop=(ic == 1))
                    if wi == 0:
                        nc.scalar.activation(out=dst[:, oc, rs], in_=ps[:, 0:256],
                                             func=AF.Identity, bias=lb_t[:, 0 + oc:1 + oc])
                    else:
                        nc.scalar.activation(out=dst[:, oc, rs], in_=ps[:, 0:256],
                                             func=AF.Copy)
            _PHASE[0] = _ph

        fT = f0
        for l in range(_nlayers):
            wqkv, wff, lb = lw[l]
            b_sa = lb[:, 0:2]
            b_sao = lb[:, 2:4]
            b_caq = lb[:, 4:6]
            b_cao = lb[:, 6:8]
            b_ff1 = lb[:, 8:16]
            b_ff2 = lb[:, 16:18]
            gp = lb[:, 18:30].rearrange("p (g f) -> p g f", g=6)

            # finish any queued fillers (layer 0's K/V from the pyramid
            # phase), then queue layer l+1's K/V for this layer's gaps
            drain(1000)
            if l + 1 < _nlayers:
                fill_q.extend(make_kv_fillers(l + 1))
                load_layer_weights(l + 1)

            # ---- self attention: QKV projections ----
            # layer 0 emits here; layers 1+ were emitted by the previous
            # layer's LN3 consumer
            if l not in qkv_pend:
                for h in range(2):
                    emit_qkv(l, wqkv, lb, fT, h)
            QTs, KTs, VTs = qkv_pend.pop(l)

            # ---- self attention: banded subtiles, software pipelined ----
            _PHASE[0] = 'sa:attn'
            OsT = act.tile([P, 2, RW], F32R, tag="OT")
            vst_tiles = {}

            def sa_prep(sti):
                qs, ql, ks = SUBTILES[sti]
                vst_ps = pms.tile([P, 256], BF16, tag="m", name=f"vtr{l}_{sti}")
                for hc in range(2):
                    nc.tensor.transpose(vst_ps[:, 128 * hc:128 * (hc + 1)],
                                        VTs[:, hc, ks:ks + 128], ident_b[:])
                vst = work.tile([P, 512], BF16, tag="vst", name=f"vst{l}_{sti}")
                vst_v = vst[:].rearrange("p (h two j) -> p h two j", h=8, two=2)
                nc.gpsimd.memset(vst_v[:, :, 1, :], 1.0)
                nc.vector.tensor_copy(vst_v[:, :, 0, :], vst_ps[:])
                vst_tiles[sti] = (vst, smask_sb[:, sti, :])

            def sa_scores(sti, p):
                qs, ql, ks = SUBTILES[sti]
                _, mskt = vst_tiles[sti]
                h0, h1 = 2 * p, 2 * p + 1
                sps = psc.tile([P, 1024], F32, tag="sc", name=f"sps{l}_{sti}_{p}")
                for hi, hh in enumerate((h0, h1)):
                    nc.tensor.matmul(sps[:, 512 * hi:512 * hi + ql],
                                     KTs[32 * (hh % 4):32 * (hh % 4) + 32, hh // 4, ks:ks + 128],
                                     QTs[32 * (hh % 4):32 * (hh % 4) + 32, hh // 4, qs:qs + ql],
                                     start=True, stop=False,
                                     tile_position=(32 * (hh % 4), 0))
                for hi in range(2):
                    nc.tensor.matmul(sps[:, 512 * hi:512 * hi + ql], ident_b[:],
                                     mskt[:, 128 * hi:128 * hi + ql],
                                     start=False, stop=True)
                spv = sps[:].rearrange("p (b q) -> p b q", b=2)[:, :, 0:ql]
                es = epool.tile([P, 256], BF16, tag="E", name=f"es{l}_{sti}_{p}")
                esv = es[:].rearrange("p (b q) -> p b q", b=2)[:, :, 0:ql]
                nc.scalar.activation(out=esv, in_=spv, func=AF.Exp)
                return es

            def sa_av(sti, p, es):
                qs, ql, ks = SUBTILES[sti]
                vst, _ = vst_tiles[sti]
                h0, h1 = 2 * p, 2 * p + 1
                avp = pav.tile([P, 512], F32, tag="av", name=f"sav{l}_{sti}_{p}")
                nc.tensor.matmul(avp[0:64, :ql], vst[:, 64 * h0:64 * h0 + 64],
                                 es[:, 0:ql], start=True, stop=True,
                                 tile_position=(0, 0))
                nc.tensor.matmul(avp[64:128, :ql], vst[:, 64 * h1:64 * h1 + 64],
                                 es[:, 128:128 + ql], start=True, stop=True,
                                 tile_position=(0, 64))
                zr = work.tile([P, 512], F32, tag="tmp", name=f"szr{l}_{sti}_{p}")
                nc.vector.reciprocal(zr[:, :ql], avp[:, :ql])
                nc.vector.tensor_mul(OsT[64 * (p % 2):64 * (p % 2) + 32, p // 2, qs:qs + ql],
                                     avp[0:32, :ql], zr[32:64, :ql])
                nc.vector.tensor_mul(OsT[64 * (p % 2) + 32:64 * (p % 2) + 64, p // 2, qs:qs + ql],
                                     avp[64:96, :ql], zr[96:128, :ql])

            sa_prep(0)
            pend = None      # (sti, p, es)
            for sti in range(len(SUBTILES)):
                if sti + 1 < len(SUBTILES):
                    sa_prep(sti + 1)
                for p in range(4):
                    es = sa_scores(sti, p)
                    if pend is not None:
                        sa_av(*pend)
                    pend = (sti, p, es)
            sa_av(*pend)
            drain(1)

            # ---- self attention out projection + LN1 ----
            _PHASE[0] = 'sa:out'
            w_sao = wqkv[:, :, 768:1024]
            R1 = state.tile([P, 2, RW], F32R, tag="fT")
            for oc in range(2):
                wsl = w_sao[:, :, 128 * oc:128 * (oc + 1)]
                ps = pms.tile([P, 512], F32, tag="m")
                for ic in range(2):
                    nc.tensor.matmul(ps[:], wsl[:, ic, :], OsT[:, ic, :],
                                     start=(ic == 0), stop=(ic == 1))
                nc.vector.scalar_tensor_tensor(out=R1[:, oc, :], in0=ps[:],
                                               scalar=b_sao[:, oc:oc + 1], in1=fT[:, oc, :],
                                               op0=OP.add, op1=OP.add)
            _PHASE[0] = 'ln1'
            f1 = state.tile([P, 2, RW], F32R, tag="fT")
            w_caq = wqkv[:, :, 1024:1280]
            QTc = act.tile([P, 2, RW], BF16, tag="QTc")

            def ln1_consume(h):
                # cross-attention Q projection for this half of the rows
                _PHASE[0] = 'ca:q'
                rs = slice(256 * h, 256 * (h + 1))
                for oc in range(2):
                    ps = pms.tile([P, 512], F32, tag="m", name=f"caq{l}_{oc}_{h}")
                    for ic in range(2):
                        nc.tensor.matmul(ps[:, 0:256],
                                         w_caq[:, ic, 128 * oc:128 * (oc + 1)],
                                         f1[:, ic, rs], start=(ic == 0), stop=(ic == 1))
                    nc.scalar.activation(out=QTc[:, oc, rs], in_=ps[:, 0:256],
                                         func=AF.Identity, bias=b_caq[:, oc:oc + 1])
                _PHASE[0] = 'ln1'

            _layernorm(nc, tc, pms, psc, work, stats, act, R1, f1, lb, 0, invn,
                       invn_b, drain, on_half=ln1_consume)

            _PHASE[0] = 'ca:attn'
            KT = kv_state[l]['KT']
            Vp = kv_state[l]['Vp']
            OcT = act.tile([P, 2, RW], F32R, tag="OT")
            for p in range(4):
                h0, h1 = 2 * p, 2 * p + 1
                # DoubleRow outputs must start at PSUM partition 0: one tile
                # per head, each using partitions 0:64 (32 v-rows + 32 ones)
                avp_a = pav.tile([P, 512], F32, tag="av", name=f"cava{l}_{p}")
                avp_b = pav.tile([P, 512], F32, tag="av", name=f"cavb{l}_{p}")

                def ca_scores(pair):
                    # exp(logit - 2): the -2 shift keeps exp within e4m3
                    # range; softmax is invariant to it (numerator and the
                    # ones-column denominator scale together).  Even pairs
                    # exp on Act -> fp8 (DoubleRow AV); odd pairs exp on DVE
                    # via the Schraudolph bit-trick in bf16 (2x DVE mode) so
                    # the two engines split the softmax load.
                    use_act = (pair % 4 != 3)
                    ec2 = epool.tile([P, 2, 1024], FP8 if use_act else BF16,
                                     tag="E" if use_act else "Eb",
                                     bufs=4 if use_act else 2,
                                     name=f"ec{l}_{p}_{pair}")
                    for j in range(2):
                        kc = 2 * pair + j
                        scp = psc.tile([P, 1024], F32, tag="sc", name=f"csc{l}_{p}_{kc}")
                        for hi, hh in enumerate((h0, h1)):
                            nc.tensor.matmul(scp[:, 512 * hi:512 * (hi + 1)],
                                             KT[32 * (hh % 4):32 * (hh % 4) + 32, hh // 4, 128 * kc:128 * (kc + 1)],
                                             QTc[32 * (hh % 4):32 * (hh % 4) + 32, hh // 4, :],
                                             start=True, stop=True, tile_position=(32 * (hh % 4), 0))
                        if use_act:
                            nc.scalar.activation(out=ec2[:, j, :], in_=scp[:],
                                                 func=AF.Exp, bias=cexp[:])
                        else:
                            # Schraudolph exp to bf16 via int16 bits:
                            # bits = s*(2^7/ln2) + (127*2^7 - 7.6 - 2*2^7/ln2)
                            nc.vector.tensor_scalar(
                                out=ec2[:, j, :].bitcast(mybir.dt.int16),
                                in0=scp[:], scalar1=184.665, scalar2=15879.07,
                                op0=OP.mult, op1=OP.add)
                    return ec2

                def ca_av(pair, ec2):
                    st_, sp_ = (pair == 0), (pair == 7)
                    kc0 = 2 * pair
                    if pair % 4 != 3:
                        # fp8 DoubleRow: two kc-chunks contracted per pass
                        e2v = ec2[:].rearrange("p two (b q) -> p two b q", b=2)
                        nc.tensor.matmul(avp_a[0:64, :], Vp[:, kc0:kc0 + 2, 64 * h0:64 * h0 + 64],
                                         e2v[:, :, 0, :], start=st_, stop=False,
                                         perf_mode=PM.DoubleRow)
                        nc.tensor.matmul(avp_b[0:64, :], Vp[:, kc0:kc0 + 2, 64 * h1:64 * h1 + 64],
                                         e2v[:, :, 1, :], start=st_, stop=False,
                                         perf_mode=PM.DoubleRow)
                    else:
                        for j in range(2):
                            kc = kc0 + j
                            last = sp_ and (j == 1)
                            nc.tensor.matmul(avp_a[0:64, :], Vp[:, kc, 64 * h0:64 * h0 + 64],
                                             ec2[:, j, 0:512], start=False, stop=last)
                            nc.tensor.matmul(avp_b[0:64, :], Vp[:, kc, 64 * h1:64 * h1 + 64],
                                             ec2[:, j, 512:1024], start=False, stop=last)

                ec_pend = ca_scores(0)
                for pair in range(1, 8):
                    ec_next = ca_scores(pair)
                    ca_av(pair - 1, ec_pend)
                    ec_pend = ec_next
                ca_av(7, ec_pend)

                zr = work.tile([P, 512], F32, tag="tmp", name=f"czr{l}_{p}")
                nc.vector.reciprocal(zr[0:32, :], avp_a[32:64, :])
                nc.vector.reciprocal(zr[32:64, :], avp_b[32:64, :])
                nc.vector.tensor_mul(OcT[64 * (p % 2):64 * (p % 2) + 32, p // 2, :],
                                     avp_a[0:32, :], zr[0:32, :])
                nc.vector.tensor_mul(OcT[64 * (p % 2) + 32:64 * (p % 2) + 64, p // 2, :],
                                     avp_b[0:32, :], zr[32:64, :])

            _PHASE[0] = 'ca:out'
            w_cao = wqkv[:, :, 1280:1536]
            R2 = state.tile([P, 2, RW], F32R, tag="fT")
            for oc in range(2):
                wsl = w_cao[:, :, 128 * oc:128 * (oc + 1)]
                ps = pms.tile([P, 512], F32, tag="m")
                for ic in range(2):
                    nc.tensor.matmul(ps[:], wsl[:, ic, :], OcT[:, ic, :],
                                     start=(ic == 0), stop=(ic == 1))
                nc.vector.scalar_tensor_tensor(out=R2[:, oc, :], in0=ps[:],
                                               scalar=b_cao[:, oc:oc + 1], in1=f1[:, oc, :],
                                               op0=OP.add, op1=OP.add)
            _PHASE[0] = 'ln2'
            f2 = state.tile([P, 2, RW], F32R, tag="fT")
            w_ff1 = wff[:, 0:2048].rearrange("p (c f) -> p c f", c=2)
            w_ff2 = wff[:, 2048:4096].rearrange("p (c f) -> p c f", c=8)
            R3 = state.tile([P, 2, RW], F32R, tag="fT")

            def ln2_consume(h):
                # full FFN for this half of the rows; h0's FFN PE work runs
                # under h1's rsqrt chain
                _PHASE[0] = 'ffn'
                rs = slice(256 * h, 256 * (h + 1))
                Hb = big.tile([P, 8, 256], F32R, tag="Hb", name=f"Hb{l}_{h}")
                for j in range(8):
                    ps = psc.tile([P, 512], F32, tag="sc", name=f"ffa{l}_{j}_{h}")
                    for ic in range(2):
                        nc.tensor.matmul(ps[:, 0:256], w_ff1[:, ic, 128 * j:128 * (j + 1)],
                                         f2[:, ic, rs], start=(ic == 0), stop=(ic == 1))
                    nc.scalar.activation(out=Hb[:, j, :], in_=ps[:, 0:256],
                                         func=AF.Relu, bias=b_ff1[:, j:j + 1])
                pool2 = pms if h == 0 else pav
                tag2 = "m" if h == 0 else "av"
                ps_oc = [pool2.tile([P, 512], F32, tag=tag2, name=f"ffp{l}_{i2}_{h}")
                         for i2 in range(2)]
                for j in range(8):
                    for oc in range(2):
                        nc.tensor.matmul(ps_oc[oc][:, 0:256],
                                         w_ff2[:, j, 128 * oc:128 * (oc + 1)],
                                         Hb[:, j, :], start=(j == 0), stop=(j == 7))
                for oc in range(2):
                    nc.vector.scalar_tensor_tensor(out=R3[:, oc, rs],
                                                   in0=ps_oc[oc][:, 0:256],
                                                   scalar=b_ff2[:, oc:oc + 1],
                                                   in1=f2[:, oc, rs],
                                                   op0=OP.add, op1=OP.add)
                _PHASE[0] = 'ln2'

            _layernorm(nc, tc, pms, psc, work, stats, act, R2, f2, lb, 1, invn,
                       invn_b, drain, on_half=ln2_consume)
            _PHASE[0] = 'ln3'
            f3 = state.tile([P, 2, RW], F32R, tag="fT")
            if l == _nlayers - 1:
                def ln3_consume(h, _f3=f3):
                    for oc in range(2):
                        nc.sync.dma_start(out=t['t_out'][:, oc, 256 * h:256 * (h + 1)],
                                          in_=_f3[:, oc, 256 * h:256 * (h + 1)])
            else:
                def ln3_consume(h, _l=l, _f3=f3):
                    wq_n, _wf_n, lb_n = lw[_l + 1]
                    emit_qkv(_l + 1, wq_n, lb_n, _f3, h)
            _layernorm(nc, tc, pms, psc, work, stats, act, R3, f3, lb, 2, invn,
                       invn_b, drain, drain_all=True, on_half=ln3_consume)
            fT = f3


def _layernorm(nc, tc, pms, psc, work, stats, act, R, out, lb, which, invn,
               invn_b, drain, drain_all=False, on_half=None):
    """Feature-major LN over d=256 (2 partition chunks), rows on free dim.
    Stats via all-(1/256) stationary matmuls producing 128-row broadcasts.
    1/sigma = rsqrt(var+eps) via the quake bit-trick seed plus one Newton
    step -- no Ln table on the Act engine (kernel stays on the exp/square
    act set, zero LoadActFuncSet switches).

    Split into two 256-row halves so the serial scalar chain pipelines:
    half-0 runs its Newton tail on DVE and finishes early; half-1's Newton
    tail runs on Pool in parallel.  `on_half(h)` emits the consumer's PE
    work for rows [256h, 256h+256) right after that half of `out` is
    written, keeping the PE busy under the other half's chain.

    Stats live in [P,1024] psc tiles: half h occupies cols [512h, 512h+256)
    so every matmul output lands on a 2KB PSUM bank boundary."""
    P = 128
    HW_ = RW // 2

    sq = act.tile([P, 2, RW], BF16, tag="sq")
    mB = psc.tile([P, 1024], F32, tag="sc", name=f"lnmB{which}")
    msB = psc.tile([P, 1024], F32, tag="sc", name=f"lnmsB{which}")

    def rows(h):
        return slice(HW_ * h, HW_ * (h + 1))

    def bview(t, h):
        return t[:, 512 * h:512 * h + HW_]

    for h in range(2):
        for oc in range(2):
            nc.scalar.activation(out=sq[:, oc, rows(h)], in_=R[:, oc, rows(h)],
                                 func=AF.Square)
        for ic in range(2):
            nc.tensor.matmul(bview(mB, h), invn[:], R[:, ic, rows(h)],
                             start=(ic == 0), stop=(ic == 1))
        for ic in range(2):
            nc.tensor.matmul(bview(msB, h), invn_b[:], sq[:, ic, rows(h)],
                             start=(ic == 0), stop=(ic == 1))

    # c = R - mB (DVE; emitted first so they run during the m2 waits)
    cs = {}
    for h in range(2):
        for oc in range(2):
            c = work.tile([P, HW_], F32, tag=f"tmp{oc}", bufs=2,
                          name=f"lnc{oc}_{h}")
            nc.vector.tensor_sub(c[:], R[:, oc, rows(h)], bview(mB, h))
            cs[(oc, h)] = c
    drain(3)

    m2 = stats.tile([P, RW], F32, tag="ln_a")
    v = stats.tile([P, RW], F32, tag="ln_v")
    s0 = stats.tile([P, RW], I32, tag="ln_i")
    y0i = stats.tile([P, RW], I32, tag="ln_y0")
    y0 = y0i[:].bitcast(F32)
    y2 = stats.tile([P, RW], F32, tag="ln_a2")
    p_ = stats.tile([P, RW], F32, tag="ln_p")
    q_ = stats.tile([P, RW], F32, tag="ln_q")
    r1 = stats.tile([P, RW], F32, tag="ln_r")

    # int seed for both halves up front on DVE (h1's Pool Newton tail can
    # then start while DVE runs h0's Newton tail)
    for h in range(2):
        rs = rows(h)
        nc.scalar.activation(out=m2[:, rs], in_=bview(mB, h), func=AF.Square)
        nc.vector.scalar_tensor_tensor(out=v[:, rs], in0=bview(msB, h),
                                       scalar=1e-5, in1=m2[:, rs],
                                       op0=OP.add, op1=OP.subtract)
        nc.vector.tensor_scalar(out=s0[:, rs], in0=v[:, rs].bitcast(I32),
                                scalar1=1, scalar2=None,
                                op0=OP.arith_shift_right)
        nc.vector.tensor_scalar(out=y0i[:, rs], in0=s0[:, rs], scalar1=-1,
                                scalar2=0x5F3759DF, op0=OP.mult, op1=OP.add)

    for h in range(2):
        rs = rows(h)
        eng = nc.vector if h == 0 else nc.gpsimd
        # Newton: r = y0 * (1.5 - 0.5 * v * y0^2)
        eng.tensor_tensor(out=y2[:, rs], in0=y0[:, rs], in1=y0[:, rs], op=OP.mult)
        eng.tensor_tensor(out=p_[:, rs], in0=y2[:, rs], in1=v[:, rs], op=OP.mult)
        eng.tensor_scalar(out=q_[:, rs], in0=p_[:, rs], scalar1=-0.5,
                          scalar2=1.5, op0=OP.mult, op1=OP.add)
        eng.tensor_tensor(out=r1[:, rs], in0=y0[:, rs], in1=q_[:, rs], op=OP.mult)
        for oc in range(2):
            # out = (R - mB) * r1 * gamma + beta  (gamma/beta per-partition)
            # h0 tail all-DVE; h1 splits oc0->Pool / oc1->DVE so both halves'
            # tails run concurrently
            aeng = nc.vector if (h == 0 or oc == 1) else nc.gpsimd
            d = work.tile([P, HW_], F32, tag=f"tmp2{oc}", bufs=2,
                          name=f"lnd{oc}_{h}")
            aeng.tensor_tensor(out=d[:], in0=cs[(oc, h)][:],
                               in1=r1[:, rs], op=OP.mult)
            aeng.tensor_scalar(out=out[:, oc, rs], in0=d[:],
                               scalar1=lb[:, 18 + 4 * which + oc:19 + 4 * which + oc],
                               scalar2=lb[:, 20 + 4 * which + oc:21 + 4 * which + oc],
                               op0=OP.mult, op1=OP.add)
        if on_half is not None:
            on_half(h)
        if h == 0:
            drain(2)
    drain(1000 if drain_all else 2)


# ---------------------------------------------------------------------------
# host side
# ---------------------------------------------------------------------------

def _sinusoidal_pe(t, d):
    pos = np.arange(t, dtype=np.float32)[:, None]
    div = np.exp(np.arange(0, d, 2, dtype=np.float32) * (-np.log(10000.0) / d))
    ang = pos * div
    pe = np.zeros((t, d), np.float32)
    pe[:, 0::2] = np.sin(ang)
    pe[:, 1::2] = np.cos(ang)
    return pe


def _concat_row_to_level(r):
    for li in range(NLV):
        if r < LVL_STARTS[li] + LVL_SIZES[li]:
            return li, r - LVL_STARTS[li]
    raise ValueError(r)


def _core_meta(c):
    w0 = OWN * c - HALO
    S = np.zeros((TOT, RW), np.float32)
    valid = np.zeros(RW, bool)
    lvl_of = np.full(RW, -1)
    pos_of = np.full(RW, -1)
    for j in range(RW):
        r = w0 + j
        if 0 <= r < TOT:
            S[r, j] = 1.0
            valid[j] = True
            lvl_of[j], pos_of[j] = _concat_row_to_level(r)
    pes = [_sinusoidal_pe(sz, COUT) for sz in LVL_SIZES]
    pe_plus = np.zeros((COUT, RW), np.float32)
    for j in range(RW):
        if valid[j]:
            pe_plus[:, j] = pes[lvl_of[j]][pos_of[j]]
    smask = np.full((5, 128, 256), -1e9, np.float32)
    for sti, (qs, ql, ks) in enumerate(SUBTILES):
        m = np.full((128, ql), -1e9, np.float32)
        for jq in range(ql):
            q = qs + jq
            for jk in range(128):
                k = ks + jk
                if k >= RW:
                    continue
                if valid[q] and valid[k]:
                    if lvl_of[q] == lvl_of[k] and abs(pos_of[q] - pos_of[k]) <= KBAND // 2:
                        m[jk, jq] = 0.0
                elif (not valid[q]) and k == q:
                    m[jk, jq] = 0.0
        smask[sti, :, 0:ql] = m
        smask[sti, :, 128:128 + ql] = m
    return S, pe_plus, smask


def _chunk_p(v):
    """[n*128] -> [128, n] partition-major."""
    v = np.asarray(v, np.float32)
    n = v.shape[0] // 128
    return v.reshape(n, 128).T.copy()


def _lhsT(w):
    """[O, I] weight -> [n_ic, 128, O] lhsT chunks (W^T chunked over I)."""
    wT = np.ascontiguousarray(np.asarray(w, np.float32).T)  # [I, O]
    I = wT.shape[0]
    return wT.reshape(I // 128, 128, wT.shape[1])


_NC_CACHE = None
LAST_EXEC_NS = None


def _get_nc():
    global _NC_CACHE
    if _NC_CACHE is None:
        _NC_CACHE = _build_nc()
    return _NC_CACHE


def _prepare_in_maps(inputs):
    inp = {k: np.asarray(v, np.float32) for k, v in inputs.items()}

    scale = 1.0 / np.sqrt(HD)
    common = {}
    common['w1x1'] = _lhsT(inp['conv_w'][:, :, 0]).astype(ml_dtypes.bfloat16)
    # wn0 packed [cc*3+tap, 128, 256]
    wn0 = np.stack([_lhsT(inp['neck_w0'][:, :, tp]) for tp in range(3)])  # [tap, cc, ...]
    common['wn0'] = wn0.transpose(1, 0, 2, 3).reshape(12, 128, 256).astype(ml_dtypes.bfloat16)
    # wnk packed [lvl*6 + cc*3 + tap, 128, 256]
    wnk = np.stack([np.stack([_lhsT(inp['neck_w'][lv][:, :, tp]) for tp in range(3)])
                    for lv in range(3)])                     # [lvl, tap, cc, ...]
    common['wnk'] = wnk.transpose(0, 2, 1, 3, 4).reshape(18, 128, 256).astype(ml_dtypes.bfloat16)
    cnb = np.zeros((128, 10), np.float32)
    cnb[:, 0:2] = _chunk_p(inp['conv_b'])
    nbs = [_chunk_p(inp['neck_b0'])] + [_chunk_p(inp['neck_b'][i]) for i in range(3)]
    for lv in range(4):
        cnb[:, 2 + 2 * lv:4 + 2 * lv] = nbs[lv]
    common['cnb'] = cnb

    wqkv, lbias = [], []
    for l in range(NLY):
        w = inp['sa_in_w'][l].copy()    # [768, 256]
        b = inp['sa_in_b'][l].copy()
        w[:COUT] *= scale
        b[:COUT] *= scale
        sa_w = _lhsT(w)                 # [2, 128, 768]
        sa_ow = _lhsT(inp['sa_out_w'][l])
        ca_qw = _lhsT(inp['ca_in_w'][l][:COUT] * scale)
        ca_ow = _lhsT(inp['ca_out_w'][l])
        wqkv.append(np.concatenate([sa_w, sa_ow, ca_qw, ca_ow], axis=-1))  # [2,128,1536]

        lbl = np.zeros((128, 30), np.float32)
        lbl[:, 0:2] = _chunk_p(b[:COUT])   # q bias only
        # fold self v bias through the out projection (softmax rows sum to 1)
        lbl[:, 2:4] = _chunk_p(inp['sa_out_b'][l] + inp['sa_out_w'][l] @ inp['sa_in_b'][l][2 * COUT:])
        lbl[:, 4:6] = _chunk_p(inp['ca_in_b'][l][:COUT] * scale)
        # fold cross v bias through the out projection
        lbl[:, 6:8] = _chunk_p(inp['ca_out_b'][l] + inp['ca_out_w'][l] @ inp['ca_in_b'][l][2 * COUT:])
        lbl[:, 8:16] = _chunk_p(inp['ff1_b'][l])
        lbl[:, 16:18] = _chunk_p(inp['ff2_b'][l])
        for wi, (g, bb) in enumerate(((inp['ln1_g'][l], inp['ln1_b'][l]),
                                      (inp['ln2_g'][l], inp['ln2_b'][l]),
                                      (inp['ln3_g'][l], inp['ln3_b'][l]))):
            lbl[:, 18 + 4 * wi:20 + 4 * wi] = _chunk_p(g)
            lbl[:, 20 + 4 * wi:22 + 4 * wi] = _chunk_p(bb)
        lbias.append(lbl)
    common['wqkv'] = np.stack(wqkv)
    common['lbias'] = np.stack(lbias)
    common['ca_kw'] = np.stack([_lhsT(inp['ca_in_w'][l][COUT:2 * COUT]) for l in range(NLY)]).astype(ml_dtypes.bfloat16)
    ca_vw = []
    for l in range(NLY):
        wT = _lhsT(inp['ca_in_w'][l][2 * COUT:])          # [2, 128, 256]
        waug = np.zeros((2, 128, 512), np.float32)
        for hh2 in range(H):
            waug[:, :, 64 * hh2:64 * hh2 + 32] = wT[:, :, 32 * hh2:32 * hh2 + 32]
        ca_vw.append(waug)
    common['ca_vw'] = np.stack(ca_vw).astype(ml_dtypes.bfloat16)
    wff = []
    for l in range(NLY):
        f1 = _lhsT(inp['ff1_w'][l])      # [2, 128, 1024]
        f2 = _lhsT(inp['ff2_w'][l])      # [8, 128, 256]
        wff.append(np.concatenate([f1.transpose(1, 0, 2).reshape(128, 2048),
                                   f2.transpose(1, 0, 2).reshape(128, 2048)], axis=-1))
    common['wff'] = np.stack(wff)        # [NLY, 128, 4096]

    metas = [_core_meta(c) for c in range(4)]
    in_maps = []
    for core in range(8):
        b, c = core // 4, core % 4
        S, pe_plus, smask = metas[c]
        xp = np.zeros((CIN, XP), np.float32)
        xp[:, 1:1 + T] = inp['x'][b]
        m = dict(common)
        m['xp'] = xp.reshape(4, 128, XP).astype(ml_dtypes.bfloat16)
        m['ssel'] = S.reshape(NBLK, 128, RW).astype(ml_dtypes.bfloat16)
        m['pe'] = pe_plus.reshape(2, 128, RW).transpose(1, 0, 2).astype(ml_dtypes.bfloat16)
        m['smask'] = smask.astype(ml_dtypes.bfloat16)
        in_maps.append(m)
    return in_maps


def kernel(**inputs):
    nc = _get_nc()
    in_maps = _prepare_in_maps(inputs)

    global LAST_EXEC_NS
    trace = bool(int(os.environ.get('KERN_TRACE', '0')))
    res = run_bass_kernel_spmd(nc, in_maps, list(range(8)), trace=trace)
    if res.exec_time_ns is not None:
        LAST_EXEC_NS = res.exec_time_ns

    out = np.zeros((B, COUT, TOT), np.float32)
    for core in range(8):
        b, c = core // 4, core % 4
        o = res.results[core]['out']          # [128, 2, RW]
        fT = o.transpose(1, 0, 2).reshape(COUT, RW)
        out[b, :, OWN * c:OWN * (c + 1)] = fT[:, HALO:HALO + OWN]
    return out


def bench(n=6, **inputs):
    """Time pure device execution with inputs pre-staged on the 8 cores."""
    import time
    import jax
    from jax.sharding import Mesh, PartitionSpec
    from jax.experimental.shard_map import shard_map
    from concourse import bass2jax, mybir as _mybir

    nc = _get_nc()
    in_maps = _prepare_in_maps(inputs)
    n_cores = 8

    bass2jax.install_neuronx_cc_hook()
    partition_name = nc.partition_id_tensor.name if nc.partition_id_tensor else None
    in_names, out_names, out_avals, zero_outs = [], [], [], []
    for alloc in nc.m.functions[0].allocations:
        if not isinstance(alloc, mybir.MemoryLocationSet):
            continue
        name = alloc.memorylocations[0].name
        if alloc.kind == "ExternalInput":
            if name != partition_name:
                in_names.append(name)
        elif alloc.kind == "ExternalOutput":
            out_names.append(name)
            shape = tuple(alloc.tensor_shape)
            dt = mybir.dt.np(alloc.dtype)
            out_avals.append(jax.core.ShapedArray(shape, dt))
            zero_outs.append(np.zeros(shape, dt))
    n_params, n_outs = len(in_names), len(out_avals)
    all_in_names = in_names + out_names + ([partition_name] if partition_name else [])

    def _body(*args):
        operands = list(args)
        if partition_name is not None:
            operands.append(bass2jax.partition_id_tensor())
        outs = bass2jax._bass_exec_p.bind(
            *operands, out_avals=tuple(out_avals), in_names=tuple(all_in_names),
            out_names=tuple(out_names), lowering_input_output_aliases=(),
            sim_require_finite=True, sim_require_nnan=True, nc=nc)
        return tuple(outs)

    devices = jax.devices()[:n_cores]
    mesh = Mesh(np.asarray(devices), ("core",))
    in_specs = (PartitionSpec("core"),) * (n_params + n_outs)
    out_specs = (PartitionSpec("core"),) * n_outs
    sharded = jax.jit(shard_map(_body, mesh=mesh, in_specs=in_specs,
                                out_specs=out_specs, check_rep=False),
                      keep_unused=True)  # NO donation so buffers are reusable
    from jax.sharding import NamedSharding
    shard = NamedSharding(mesh, PartitionSpec("core"))
    concat_in = [np.concatenate([np.asarray(in_maps[c][nm]) for c in range(n_cores)], axis=0)
                 for nm in in_names]
    concat_zeros = [np.zeros((n_cores * z.shape[0], *z.shape[1:]), z.dtype) for z in zero_outs]
    dev_in = [jax.device_put(a, shard) for a in concat_in]
    dev_zero = [jax.device_put(a, shard) for a in concat_zeros]
    for a in dev_in + dev_zero:
        a.block_until_ready()
    # warmup
    outs = sharded(*dev_in, *dev_zero)
    jax.block_until_ready(outs)
    times = []
    for _ in range(n):
        t0 = time.perf_counter()
        outs = sharded(*dev_in, *dev_zero)
        jax.block_until_ready(outs)
        times.append(time.perf_counter() - t0)
    return times


def timeline_estimate():
    """Cost-model single-core timeline estimate (ns)."""
    from concourse.timeline_sim import TimelineSim
    nc = _get_nc()
    ts = TimelineSim(nc, trace=False)
    ts.simulate()
    return ts

